# revision 16
# baseline (speedup 1.0000x reference)
"""Causal self-attention (QKV GEMM + RoPE + causal softmax attention + output
projection) for Trainium2, sharded over 8 NeuronCores.

Sharding: tensor-parallel over heads (2 heads/core). Each core computes the
QKV projections for its heads (full token range), RoPE, causal attention, and
a partial output projection over its heads' channels; the host sums the 8
partial projections (the only cross-core reduction) and reshapes.

Matmul operands are fp16 (full-rate PE with hidden weight loads); all
accumulation is fp32 in PSUM, softmax statistics are fp32.

Perf notes vs the first working version (490 us):
- all matmul operands converted to fp16 on the host: halves DMA traffic,
  and the x operand is laid out in DMA-issue order so every load is one
  contiguous chunk per partition (128 big descriptors instead of 1024
  small ones).
- softmax denominators accumulated on the Vector engine (at-tile adds)
  with a single [1,NQ] PE matmul per query chunk, instead of one
  full-stream [1,512] PE matmul per key tile (-45 us of PE time).
- qkv weights split into 4 independently-tracked tiles so the first
  GEMM starts as soon as the first quarter lands.
- output partials stored as fp16 (halves write-out traffic; phase C was
  DMA-bound), host accumulates in fp32.
- const/output DMAs issued from the idle SP queue.
"""

import os
import sys

import numpy as np


def _ensure_concourse():
    try:
        import concourse.bass  # noqa: F401
        return
    except ImportError:
        pass
    for p in (
        "/opt/trn_rl_repo",
        os.path.expanduser("~/.axon_site/_ro/trn_rl_repo"),
        "/root/.axon_site/_ro/trn_rl_repo",
    ):
        if os.path.isdir(p) and p not in sys.path:
            sys.path.insert(0, p)
    import concourse.bass  # noqa: F401


# Problem shape (hardcoded per contract)
B, T, C, H = 2, 2048, 2048, 16
D, RD = 128, 64
NCORES = 8
HPC = H // NCORES          # heads per core = 2
BT = B * T                 # 4096
P = 128
MT = T // P                # 16 token tiles per batch
KTC = C // P               # 16 contraction tiles over C
KH = KTC // 2              # 8 contraction tiles per half
FPC = 3 * HPC * D          # 768 qkv features per core
NQ = 512                   # query chunk
NJ = T // NQ               # 4 query chunks per instance
GROUPS = [3, 3, 3, 3, 3, 1]  # m-tile grouping in phase A
SCALE = 1.0 / float(np.sqrt(D))

_PROGRAM = None


def _build_program():
    _ensure_concourse()
    from contextlib import ExitStack

    import concourse.bacc as bacc
    import concourse.mybir as mybir
    import concourse.tile as tile
    from concourse.alu_op_type import AluOpType
    from concourse.masks import make_identity

    F32 = mybir.dt.float32
    MMDT = mybir.dt.float16
    EXP = mybir.ActivationFunctionType.Exp
    MUL = AluOpType.mult
    SUB = AluOpType.subtract
    ADD = AluOpType.add
    PSUM = "PSUM"

    nc = bacc.Bacc("TRN2", target_bir_lowering=False, debug=False,
                   num_devices=NCORES)

    # x in DMA-issue order: [p, (b, group, half, k8, tok)] contiguous chunks
    xt_d = nc.dram_tensor("xt", [P, BT * KTC], MMDT, kind="ExternalInput").ap()
    w_d = nc.dram_tensor("wqkv", [P, KTC * FPC], MMDT, kind="ExternalInput").ap()
    cos_d = nc.dram_tensor("cosw", [P, (BT // P) * RD], F32, kind="ExternalInput").ap()
    sin_d = nc.dram_tensor("sinw", [P, (BT // P) * RD], F32, kind="ExternalInput").ap()
    msk_d = nc.dram_tensor("maskd", [P, P], MMDT, kind="ExternalInput").ap()
    wp_d = nc.dram_tensor("wproj", [P, HPC * C], MMDT, kind="ExternalInput").ap()
    out_d = nc.dram_tensor("outp", [BT, C], MMDT, kind="ExternalOutput").ap()

    WQ = KTC * FPC // 4        # qkv weight quarter, 4 k-tiles each

    with tile.TileContext(nc) as tc, ExitStack() as gctx:
        ep = gctx.enter_context

        const = ep(tc.tile_pool(name="const", bufs=1))
        msk_sb = const.tile([P, P], MMDT, tag="msk")
        cos_sb = const.tile([P, (BT // P) * RD], F32, tag="cos")
        sin_sb = const.tile([P, (BT // P) * RD], F32, tag="sin")
        ident = const.tile([P, P], MMDT, tag="ident")
        ones_cf = const.tile([P, 1], F32, tag="ones_cf")
        ones_c = const.tile([P, 1], MMDT, tag="ones_c")
        wp_sb = const.tile([P, HPC * C], MMDT, tag="wp")

        # consts on the idle SP queue, off the critical path
        nc.sync.dma_start(out=msk_sb[:], in_=msk_d)
        nc.sync.dma_start(out=cos_sb[:], in_=cos_d)
        nc.sync.dma_start(out=sin_sb[:], in_=sin_d)
        nc.sync.dma_start(out=wp_sb[:], in_=wp_d)
        make_identity(nc, ident[:])
        nc.vector.memset(ones_cf[:], 1.0)
        nc.vector.tensor_copy(ones_c[:], ones_cf[:])

        qkt_pool = ep(tc.tile_pool(name="qkt", bufs=1))
        v_pool = ep(tc.tile_pool(name="v", bufs=1))
        yt_pool = ep(tc.tile_pool(name="yt", bufs=1))
        yt_all = yt_pool.tile([P, B * HPC * T], MMDT, tag="yt")
        xcol = ep(tc.tile_pool(name="xcol", bufs=2))
        rotp = ep(tc.tile_pool(name="rot", bufs=3))
        tmpp = ep(tc.tile_pool(name="tmp", bufs=2))

        # x chunk offsets in DMA-issue order
        xoffs = {}
        off = 0
        for b in range(B):
            for gi, g in enumerate(GROUPS):
                for half in range(2):
                    xoffs[(b, gi, half)] = (off, g)
                    off += KH * P * g
        prefetched = {}

        def fetch_x(b, gi, half):
            key = (b, gi, half)
            if key in prefetched:
                return prefetched.pop(key)
            xo, g = xoffs[key]
            xc = xcol.tile([P, KH, P * g], MMDT, tag="xc")
            nc.gpsimd.dma_start(
                out=xc[:],
                in_=xt_d[:, xo:xo + KH * P * g].rearrange(
                    "p (k t) -> p k t", k=KH))
            return xc

        def prefetch_x(b, gi, half):
            prefetched[(b, gi, half)] = fetch_x(b, gi, half)

        # first x chunk ahead of the weights: it lands last on the start
        # critical path
        prefetch_x(0, 0, 0)

        wstack = ExitStack()
        wpool = wstack.enter_context(tc.tile_pool(name="wqkv", bufs=1))
        w_sbs = [wpool.tile([P, WQ], MMDT, tag=f"w{q}", name=f"w{q}")
                 for q in range(4)]
        # qkv weights gate the first GEMM; quarters tracked independently
        # so kt 0-3 can start before the rest lands
        for q in range(4):
            nc.gpsimd.dma_start(out=w_sbs[q][:], in_=w_d[:, q * WQ:(q + 1) * WQ])

        def wslice(kt, lo, hi):
            return w_sbs[kt // 4][:, (kt % 4) * FPC + lo:(kt % 4) * FPC + hi]

        for b in range(B):
            qkT = qkt_pool.tile([P, 4 * T], MMDT, tag="qkT")
            v_sb = v_pool.tile([P, MT * HPC * D], MMDT, tag="v")

            # ---- Phase A: QKV GEMM + RoPE + transpose of Q,K ----
            with ExitStack() as actx:
                ap = actx.enter_context
                ps5 = ap(tc.tile_pool(name="ps5", bufs=3, space=PSUM))
                ps2 = ap(tc.tile_pool(name="ps2", bufs=3, space=PSUM))
                pst = ap(tc.tile_pool(name="pst", bufs=2, space=PSUM))

                m0 = 0
                for gi, g in enumerate(GROUPS):
                    p5s = [ps5.tile([P, 512], F32, tag="p5", name=f"p5_{b}_{m0}_{i}")
                           for i in range(g)]
                    p2s = [ps2.tile([P, 256], F32, tag="p2", name=f"p2_{b}_{m0}_{i}")
                           for i in range(g)]
                    for half in range(2):
                        xc = fetch_x(b, gi, half)
                        for mi in range(g):
                            for k8 in range(KH):
                                kt = half * KH + k8
                                lhsT = xc[:, k8, mi * P:(mi + 1) * P]
                                nc.tensor.matmul(
                                    p5s[mi][:], lhsT, wslice(kt, 0, 512),
                                    start=(kt == 0), stop=(kt == KTC - 1))
                                nc.tensor.matmul(
                                    p2s[mi][:], lhsT, wslice(kt, 512, FPC),
                                    start=(kt == 0), stop=(kt == KTC - 1))
                    for mi in range(g):
                        m = m0 + mi
                        gm = b * MT + m
                        p5 = p5s[mi]
                        p2 = p2s[mi]
                        # RoPE on the q|k half (psum chunk p5), writes rot
                        rot = rotp.tile([P, 512], MMDT, tag="rot")
                        p3 = p5[:].rearrange("p (blk two d) -> p blk two d",
                                             two=2, d=RD)
                        re_, im_ = p3[:, :, 0, :], p3[:, :, 1, :]
                        r3 = rot[:].rearrange("p (blk two d) -> p blk two d",
                                              two=2, d=RD)
                        cosb = (cos_sb[:, gm * RD:(gm + 1) * RD]
                                .unsqueeze(1).broadcast_to([P, 4, RD]))
                        sinb = (sin_sb[:, gm * RD:(gm + 1) * RD]
                                .unsqueeze(1).broadcast_to([P, 4, RD]))
                        t1 = tmpp.tile([P, 256], F32, tag="t1")
                        t2 = tmpp.tile([P, 256], F32, tag="t2")
                        t1v = t1[:].rearrange("p (blk d) -> p blk d", d=RD)
                        t2v = t2[:].rearrange("p (blk d) -> p blk d", d=RD)
                        nc.vector.tensor_tensor(t1v, re_, cosb, MUL)
                        nc.vector.tensor_tensor(t2v, im_, sinb, MUL)
                        nc.vector.tensor_tensor(r3[:, :, 0, :], t1v, t2v, SUB)
                        t3 = tmpp.tile([P, 256], F32, tag="t3")
                        t4 = tmpp.tile([P, 256], F32, tag="t4")
                        t3v = t3[:].rearrange("p (blk d) -> p blk d", d=RD)
                        t4v = t4[:].rearrange("p (blk d) -> p blk d", d=RD)
                        nc.vector.tensor_tensor(t3v, re_, sinb, MUL)
                        nc.vector.tensor_tensor(t4v, im_, cosb, MUL)
                        nc.vector.tensor_tensor(r3[:, :, 1, :], t3v, t4v, ADD)
                        # V eviction
                        nc.scalar.copy(v_sb[:, m * HPC * D:(m + 1) * HPC * D],
                                       p2[:])
                        # Transpose rotated q,k into [d, t] layout
                        for hb in range(4):
                            tp = pst.tile([P, P], MMDT, tag="tp")
                            nc.tensor.transpose(
                                tp[:], rot[:, hb * P:(hb + 1) * P], ident[:])
                            nc.scalar.copy(
                                qkT[:, hb * T + m * P:(hb * T + (m + 1) * P)],
                                tp[:])
                    m0 += g

            # prefetch next batch's first x chunks so phase A(b+1) starts
            # without a DMA stall
            if b + 1 < B:
                prefetch_x(b + 1, 0, 0)
                prefetch_x(b + 1, 0, 1)

            # ---- Phase B: causal attention per head ----
            with ExitStack() as bctx:
                bp = bctx.enter_context
                attnp = bp(tc.tile_pool(name="attn", bufs=6))
                saccp = bp(tc.tile_pool(name="sacc", bufs=2))
                rcpp = bp(tc.tile_pool(name="rcp", bufs=2))
                repp = bp(tc.tile_pool(name="rep", bufs=2))
                pss = bp(tc.tile_pool(name="pss", bufs=4, space=PSUM))
                psy = bp(tc.tile_pool(name="psy", bufs=2, space=PSUM))
                psm = bp(tc.tile_pool(name="psm", bufs=2, space=PSUM))

                def finalize_a(inst, j, y_ps, s_ps):
                    # stage 1 of the deferred softmax normalization: recip
                    # on DVE, then replicate on Pool. Emitted a few tiles
                    # ahead of stage 2 so the Pool broadcast latency is
                    # hidden behind DVE work instead of stalling it.
                    rcp = rcpp.tile([1, NQ], F32, tag="rc",
                                    name=f"rc_{inst}_{j}")
                    with nc.allow_low_precision(reason="softmax recip"):
                        nc.vector.reciprocal_approx_fast(out=rcp[:],
                                                         in_=s_ps[:])
                    reps = repp.tile([P, NQ], F32, tag="rs",
                                     name=f"rs_{inst}_{j}")
                    nc.gpsimd.partition_broadcast(reps[:], rcp[:], channels=P)
                    return reps

                def finalize_b(inst, j, y_ps, reps):
                    nc.vector.tensor_tensor(
                        yt_all[:, inst * T + j * NQ: inst * T + (j + 1) * NQ],
                        y_ps[:], reps[:], MUL)

                pending = None
                for h in range(HPC):
                    inst = b * HPC + h
                    for j in range(NJ):
                        y_ps = psy.tile([P, NQ], F32, tag="y",
                                        name=f"y_{inst}_{j}")
                        s_ps = psm.tile([1, NQ], F32, tag="s",
                                        name=f"s_{inst}_{j}")
                        sacc = saccp.tile([P, NQ], MMDT, tag="sa",
                                          name=f"sa_{inst}_{j}")
                        nkt = 4 * (j + 1)
                        for kt in range(nkt):
                            # diagonal key tiles: columns < qlo are fully
                            # masked; skip them in every op (causal
                            # narrowing saves ~15% of exp/adds/matmul)
                            ktl = kt - (nkt - 4)
                            qlo = max(ktl, 0) * P
                            sc = pss.tile([P, NQ], F32, tag="sc",
                                          name=f"sc_{inst}_{j}_{kt}")
                            nc.tensor.matmul(
                                sc[:, qlo:],
                                qkT[:, (2 + h) * T + kt * P:
                                    (2 + h) * T + (kt + 1) * P],
                                qkT[:, h * T + j * NQ + qlo:
                                    h * T + (j + 1) * NQ],
                                start=True, stop=True)
                            at = attnp.tile([P, NQ], MMDT, tag="at",
                                            name=f"at_{inst}_{j}_{kt}")
                            nc.scalar.activation(at[:, qlo:], sc[:, qlo:],
                                                 EXP, scale=SCALE)
                            if ktl >= 0:
                                # strict-triangle mask on the one partially
                                # valid 128-col subtile
                                nc.vector.tensor_tensor(
                                    at[:, qlo:qlo + P], at[:, qlo:qlo + P],
                                    msk_sb[:], MUL)
                            # denominator partials on DVE (fp16 lanes, each
                            # accumulates <=16 exp values -> no overflow)
                            if kt == 0:
                                nc.vector.tensor_copy(sacc[:], at[:])
                            else:
                                nc.vector.tensor_tensor(sacc[:, qlo:],
                                                        sacc[:, qlo:],
                                                        at[:, qlo:], ADD)
                            nc.tensor.matmul(
                                y_ps[:, qlo:],
                                v_sb[:, kt * HPC * D + h * D:
                                     kt * HPC * D + (h + 1) * D],
                                at[:, qlo:], start=(kt == 0),
                                stop=(kt == nkt - 1), skip_group_check=True)
                            if kt == 1 and pending is not None:
                                pending = (*pending[:3],
                                           finalize_a(*pending))
                            if kt == 3 and pending is not None:
                                finalize_b(*pending)
                                pending = None
                        # single partition-sum matmul per query chunk
                        nc.tensor.matmul(s_ps[:], ones_c[:], sacc[:],
                                         start=True, stop=True)
                        pending = (inst, j, y_ps, s_ps)
                if pending is not None:
                    finalize_b(*pending[:3], finalize_a(*pending))
                    pending = None

        wstack.close()

        # ---- Phase C: partial output projection ----
        with ExitStack() as cctx:
            cp = cctx.enter_context
            outrow = cp(tc.tile_pool(name="orow", bufs=3))
            pso = cp(tc.tile_pool(name="pso", bufs=8, space=PSUM))
            for b in range(B):
                for m in range(MT):
                    orow = outrow.tile([P, C], MMDT, tag="orow")
                    for oc in range(4):
                        op = pso.tile([P, 512], F32, tag="op")
                        for h in range(HPC):
                            nc.tensor.matmul(
                                op[:],
                                yt_all[:, (b * HPC + h) * T + m * P:
                                       (b * HPC + h) * T + (m + 1) * P],
                                wp_sb[:, h * C + oc * 512: h * C + (oc + 1) * 512],
                                start=(h == 0), stop=(h == HPC - 1))
                        if oc % 2 == 0:
                            nc.scalar.copy(orow[:, oc * 512:(oc + 1) * 512], op[:])
                        else:
                            nc.vector.tensor_copy(
                                orow[:, oc * 512:(oc + 1) * 512], op[:])
                    nc.sync.dma_start(
                        out=out_d[(b * MT + m) * P:(b * MT + m + 1) * P, :],
                        in_=orow[:])

    nc.compile()
    return nc


def _perm(rows):
    return np.concatenate([rows[0::2], rows[1::2]], axis=0)


def _host_inputs(x, mask, freqs_cos, freqs_sin, w_attn, w_proj):
    f32 = np.float32
    f16 = np.float16
    x = np.asarray(x, f32)
    mask = np.asarray(mask)
    fc = np.asarray(freqs_cos, f32)
    fs = np.asarray(freqs_sin, f32)
    w_attn = np.asarray(w_attn, f32)
    w_proj = np.asarray(w_proj, f32)

    # x in DMA-issue order: per partition, contiguous [b][group][half][k8][tok]
    Xv = x.reshape(B, T, KTC, P).transpose(3, 0, 2, 1)  # [p, b, kt, t]
    chunks = []
    for b in range(B):
        m0 = 0
        for g in GROUPS:
            blk = Xv[:, b, :, m0 * P:(m0 + g) * P]       # [p, 16, g*128]
            chunks.append(blk.reshape(P, -1))
            m0 += g
    xt_host = np.ascontiguousarray(np.concatenate(chunks, axis=1)).astype(f16)

    def rows_arrange(a):  # [BT, RD] -> [P, (BT//P)*RD]
        return np.ascontiguousarray(
            a.reshape(BT // P, P, RD).transpose(1, 0, 2).reshape(P, -1))

    cosw = rows_arrange(np.concatenate([fc] * B, axis=0))
    sinw = rows_arrange(np.concatenate([fs] * B, axis=0))

    # one [k, q] triangle (attend iff k <= q) covers every diagonal subtile
    maskd = np.ascontiguousarray(np.triu(np.ones((P, P), dtype=f16)))

    wq, wk, wv = w_attn[0:C], w_attn[C:2 * C], w_attn[2 * C:3 * C]
    in_maps = []
    for c in range(NCORES):
        h0, h1 = HPC * c, HPC * c + 1
        Wc = np.concatenate([
            _perm(wq[h0 * D:(h0 + 1) * D]), _perm(wq[h1 * D:(h1 + 1) * D]),
            _perm(wk[h0 * D:(h0 + 1) * D]), _perm(wk[h1 * D:(h1 + 1) * D]),
            wv[h0 * D:(h0 + 1) * D], wv[h1 * D:(h1 + 1) * D]], axis=0)
        wqkv_c = np.ascontiguousarray(
            Wc.T.reshape(KTC, P, FPC).transpose(1, 0, 2).reshape(P, KTC * FPC)
        ).astype(f16)
        wp_c = w_proj[:, c * HPC * D:(c + 1) * HPC * D].T  # [256, C]
        wp_c = np.ascontiguousarray(
            wp_c.reshape(HPC, P, C).transpose(1, 0, 2).reshape(P, HPC * C)
        ).astype(f16)
        in_maps.append({
            "xt": xt_host, "wqkv": wqkv_c, "cosw": cosw, "sinw": sinw,
            "maskd": maskd, "wproj": wp_c,
        })
    return in_maps


def kernel(x, mask, freqs_cos, freqs_sin, w_attn, w_proj):
    global _PROGRAM
    _ensure_concourse()
    from concourse.bass_utils import run_bass_kernel_spmd

    if _PROGRAM is None:
        _PROGRAM = _build_program()
    nc = _PROGRAM

    in_maps = _host_inputs(x, mask, freqs_cos, freqs_sin, w_attn, w_proj)
    res = run_bass_kernel_spmd(nc, in_maps, list(range(NCORES)))
    out = res.results[0]["outp"].astype(np.float32)
    for i in range(1, NCORES):
        out = out + res.results[i]["outp"].astype(np.float32)
    return np.ascontiguousarray(out.reshape(B, T, C))


# revision 17
# speedup vs baseline: 1.0624x; 1.0624x over previous
"""Causal self-attention (QKV GEMM + RoPE + causal softmax attention + output
projection) for Trainium2, sharded over 8 NeuronCores.

Sharding: tensor-parallel over heads (2 heads/core). Each core computes the
QKV projections for its heads (full token range), RoPE, causal attention, and
a partial output projection over its heads' channels; the host sums the 8
partial projections (the only cross-core reduction) and reshapes.

Matmul operands are fp16 (full-rate PE with hidden weight loads); all
accumulation is fp32 in PSUM, softmax statistics are fp32.

Perf notes vs the first working version (490 us):
- all matmul operands converted to fp16 on the host: halves DMA traffic,
  and the x operand is laid out in DMA-issue order so every load is one
  contiguous chunk per partition (128 big descriptors instead of 1024
  small ones).
- softmax denominators accumulated on the Vector engine (at-tile adds)
  with a single [1,NQ] PE matmul per query chunk, instead of one
  full-stream [1,512] PE matmul per key tile (-45 us of PE time).
- qkv weights split into 4 independently-tracked tiles so the first
  GEMM starts as soon as the first quarter lands.
- output partials stored as fp16 (halves write-out traffic; phase C was
  DMA-bound), host accumulates in fp32.
- const/output DMAs issued from the idle SP queue.
"""

import os
import sys

import numpy as np


def _ensure_concourse():
    try:
        import concourse.bass  # noqa: F401
        return
    except ImportError:
        pass
    for p in (
        "/opt/trn_rl_repo",
        os.path.expanduser("~/.axon_site/_ro/trn_rl_repo"),
        "/root/.axon_site/_ro/trn_rl_repo",
    ):
        if os.path.isdir(p) and p not in sys.path:
            sys.path.insert(0, p)
    import concourse.bass  # noqa: F401


# Problem shape (hardcoded per contract)
B, T, C, H = 2, 2048, 2048, 16
D, RD = 128, 64
NCORES = 8
HPC = H // NCORES          # heads per core = 2
BT = B * T                 # 4096
P = 128
MT = T // P                # 16 token tiles per batch
KTC = C // P               # 16 contraction tiles over C
KH = KTC // 2              # 8 contraction tiles per half
FPC = 3 * HPC * D          # 768 qkv features per core
NQ = 512                   # query chunk
NJ = T // NQ               # 4 query chunks per instance
GROUPS = [3, 3, 3, 3, 3, 1]  # m-tile grouping in phase A
SCALE = 1.0 / float(np.sqrt(D))

_PROGRAM = None


def _build_program():
    _ensure_concourse()
    from contextlib import ExitStack

    import concourse.bacc as bacc
    import concourse.mybir as mybir
    import concourse.tile as tile
    from concourse.alu_op_type import AluOpType
    from concourse.masks import make_identity

    F32 = mybir.dt.float32
    MMDT = mybir.dt.float16
    EXP = mybir.ActivationFunctionType.Exp
    MUL = AluOpType.mult
    SUB = AluOpType.subtract
    ADD = AluOpType.add
    PSUM = "PSUM"

    nc = bacc.Bacc("TRN2", target_bir_lowering=False, debug=False,
                   num_devices=NCORES)

    # x in DMA-issue order: [p, (b, group, half, k8, tok)] contiguous chunks
    xt_d = nc.dram_tensor("xt", [P, BT * KTC], MMDT, kind="ExternalInput").ap()
    w_d = nc.dram_tensor("wqkv", [P, KTC * FPC], MMDT, kind="ExternalInput").ap()
    cos_d = nc.dram_tensor("cosw", [P, (BT // P) * RD], F32, kind="ExternalInput").ap()
    sin_d = nc.dram_tensor("sinw", [P, (BT // P) * RD], F32, kind="ExternalInput").ap()
    msk_d = nc.dram_tensor("maskd", [P, P], MMDT, kind="ExternalInput").ap()
    wp_d = nc.dram_tensor("wproj", [P, HPC * C], MMDT, kind="ExternalInput").ap()
    out_d = nc.dram_tensor("outp", [BT, C], MMDT, kind="ExternalOutput").ap()

    WQ = KTC * FPC // 4        # qkv weight quarter, 4 k-tiles each

    with tile.TileContext(nc) as tc, ExitStack() as gctx:
        ep = gctx.enter_context

        const = ep(tc.tile_pool(name="const", bufs=1))
        msk_sb = const.tile([P, P], MMDT, tag="msk")
        cos_sb = const.tile([P, (BT // P) * RD], F32, tag="cos")
        sin_sb = const.tile([P, (BT // P) * RD], F32, tag="sin")
        ident = const.tile([P, P], MMDT, tag="ident")
        ones_cf = const.tile([P, 1], F32, tag="ones_cf")
        ones_c = const.tile([P, 1], MMDT, tag="ones_c")
        wp_sb = const.tile([P, HPC * C], MMDT, tag="wp")

        # consts on the idle SP queue, off the critical path
        nc.sync.dma_start(out=msk_sb[:], in_=msk_d)
        nc.sync.dma_start(out=cos_sb[:], in_=cos_d)
        nc.sync.dma_start(out=sin_sb[:], in_=sin_d)
        nc.sync.dma_start(out=wp_sb[:], in_=wp_d)
        make_identity(nc, ident[:])
        nc.vector.memset(ones_cf[:], 1.0)
        nc.vector.tensor_copy(ones_c[:], ones_cf[:])

        qkt_pool = ep(tc.tile_pool(name="qkt", bufs=1))
        v_pool = ep(tc.tile_pool(name="v", bufs=1))
        yt_pool = ep(tc.tile_pool(name="yt", bufs=1))
        yt_all = yt_pool.tile([P, B * HPC * T], MMDT, tag="yt")
        xcol = ep(tc.tile_pool(name="xcol", bufs=2))
        rotp = ep(tc.tile_pool(name="rot", bufs=3))
        tmpp = ep(tc.tile_pool(name="tmp", bufs=2))

        # x chunk offsets in DMA-issue order
        xoffs = {}
        off = 0
        for b in range(B):
            for gi, g in enumerate(GROUPS):
                for half in range(2):
                    xoffs[(b, gi, half)] = (off, g)
                    off += KH * P * g
        prefetched = {}

        def fetch_x(b, gi, half):
            key = (b, gi, half)
            if key in prefetched:
                return prefetched.pop(key)
            xo, g = xoffs[key]
            xc = xcol.tile([P, KH, P * g], MMDT, tag="xc")
            nc.gpsimd.dma_start(
                out=xc[:],
                in_=xt_d[:, xo:xo + KH * P * g].rearrange(
                    "p (k t) -> p k t", k=KH))
            return xc

        def prefetch_x(b, gi, half):
            prefetched[(b, gi, half)] = fetch_x(b, gi, half)

        # first x chunk ahead of the weights: it lands last on the start
        # critical path
        prefetch_x(0, 0, 0)

        wstack = ExitStack()
        wpool = wstack.enter_context(tc.tile_pool(name="wqkv", bufs=1))
        w_sbs = [wpool.tile([P, WQ], MMDT, tag=f"w{q}", name=f"w{q}")
                 for q in range(4)]
        # qkv weights gate the first GEMM; quarters tracked independently
        # so kt 0-3 can start before the rest lands
        for q in range(4):
            nc.gpsimd.dma_start(out=w_sbs[q][:], in_=w_d[:, q * WQ:(q + 1) * WQ])

        def wslice(kt, lo, hi):
            return w_sbs[kt // 4][:, (kt % 4) * FPC + lo:(kt % 4) * FPC + hi]

        for b in range(B):
            qkT = qkt_pool.tile([P, 4 * T], MMDT, tag="qkT")
            v_sb = v_pool.tile([P, MT * HPC * D], MMDT, tag="v")

            # ---- Phase A: QKV GEMM + RoPE + transpose of Q,K ----
            with ExitStack() as actx:
                ap = actx.enter_context
                ps5 = ap(tc.tile_pool(name="ps5", bufs=3, space=PSUM))
                ps2 = ap(tc.tile_pool(name="ps2", bufs=3, space=PSUM))
                pst = ap(tc.tile_pool(name="pst", bufs=2, space=PSUM))

                m0 = 0
                for gi, g in enumerate(GROUPS):
                    p5s = [ps5.tile([P, 512], F32, tag="p5", name=f"p5_{b}_{m0}_{i}")
                           for i in range(g)]
                    p2s = [ps2.tile([P, 256], F32, tag="p2", name=f"p2_{b}_{m0}_{i}")
                           for i in range(g)]
                    for half in range(2):
                        xc = fetch_x(b, gi, half)
                        for mi in range(g):
                            for k8 in range(KH):
                                kt = half * KH + k8
                                lhsT = xc[:, k8, mi * P:(mi + 1) * P]
                                nc.tensor.matmul(
                                    p5s[mi][:], lhsT, wslice(kt, 0, 512),
                                    start=(kt == 0), stop=(kt == KTC - 1))
                                nc.tensor.matmul(
                                    p2s[mi][:], lhsT, wslice(kt, 512, FPC),
                                    start=(kt == 0), stop=(kt == KTC - 1))
                    for mi in range(g):
                        m = m0 + mi
                        gm = b * MT + m
                        p5 = p5s[mi]
                        p2 = p2s[mi]
                        # RoPE on the q|k half (psum chunk p5), writes rot
                        rot = rotp.tile([P, 512], MMDT, tag="rot")
                        p3 = p5[:].rearrange("p (blk two d) -> p blk two d",
                                             two=2, d=RD)
                        re_, im_ = p3[:, :, 0, :], p3[:, :, 1, :]
                        r3 = rot[:].rearrange("p (blk two d) -> p blk two d",
                                              two=2, d=RD)
                        cosb = (cos_sb[:, gm * RD:(gm + 1) * RD]
                                .unsqueeze(1).broadcast_to([P, 4, RD]))
                        sinb = (sin_sb[:, gm * RD:(gm + 1) * RD]
                                .unsqueeze(1).broadcast_to([P, 4, RD]))
                        t1 = tmpp.tile([P, 256], F32, tag="t1")
                        t2 = tmpp.tile([P, 256], F32, tag="t2")
                        t1v = t1[:].rearrange("p (blk d) -> p blk d", d=RD)
                        t2v = t2[:].rearrange("p (blk d) -> p blk d", d=RD)
                        nc.vector.tensor_tensor(t1v, re_, cosb, MUL)
                        nc.vector.tensor_tensor(t2v, im_, sinb, MUL)
                        nc.vector.tensor_tensor(r3[:, :, 0, :], t1v, t2v, SUB)
                        t3 = tmpp.tile([P, 256], F32, tag="t3")
                        t4 = tmpp.tile([P, 256], F32, tag="t4")
                        t3v = t3[:].rearrange("p (blk d) -> p blk d", d=RD)
                        t4v = t4[:].rearrange("p (blk d) -> p blk d", d=RD)
                        nc.vector.tensor_tensor(t3v, re_, sinb, MUL)
                        nc.vector.tensor_tensor(t4v, im_, cosb, MUL)
                        nc.vector.tensor_tensor(r3[:, :, 1, :], t3v, t4v, ADD)
                        # V eviction
                        nc.scalar.copy(v_sb[:, m * HPC * D:(m + 1) * HPC * D],
                                       p2[:])
                        # Transpose rotated q,k into [d, t] layout
                        for hb in range(4):
                            tp = pst.tile([P, P], MMDT, tag="tp")
                            nc.tensor.transpose(
                                tp[:], rot[:, hb * P:(hb + 1) * P], ident[:])
                            nc.scalar.copy(
                                qkT[:, hb * T + m * P:(hb * T + (m + 1) * P)],
                                tp[:])
                    m0 += g

            # prefetch next batch's first x chunks so phase A(b+1) starts
            # without a DMA stall
            if b + 1 < B:
                prefetch_x(b + 1, 0, 0)
                prefetch_x(b + 1, 0, 1)

            # ---- Phase B: causal attention per head ----
            with ExitStack() as bctx:
                bp = bctx.enter_context
                attnp = bp(tc.tile_pool(name="attn", bufs=4))
                saccp = bp(tc.tile_pool(name="sacc", bufs=2))
                rcpp = bp(tc.tile_pool(name="rcp", bufs=2))
                repp = bp(tc.tile_pool(name="rep", bufs=2))
                pss = bp(tc.tile_pool(name="pss", bufs=4, space=PSUM))
                psy = bp(tc.tile_pool(name="psy", bufs=2, space=PSUM))
                psm = bp(tc.tile_pool(name="psm", bufs=2, space=PSUM))

                def finalize_a(inst, j, y_ps, s_ps):
                    # stage 1 of the deferred softmax normalization: recip
                    # on DVE, then replicate on Pool. Emitted a few tiles
                    # ahead of stage 2 so the Pool broadcast latency is
                    # hidden behind DVE work instead of stalling it.
                    rcp = rcpp.tile([1, NQ], F32, tag="rc",
                                    name=f"rc_{inst}_{j}")
                    with nc.allow_low_precision(reason="softmax recip"):
                        nc.vector.reciprocal_approx_fast(out=rcp[:],
                                                         in_=s_ps[:])
                    reps = repp.tile([P, NQ], F32, tag="rs",
                                     name=f"rs_{inst}_{j}")
                    nc.gpsimd.partition_broadcast(reps[:], rcp[:], channels=P)
                    return reps

                def finalize_b(inst, j, y_ps, reps):
                    nc.vector.tensor_tensor(
                        yt_all[:, inst * T + j * NQ: inst * T + (j + 1) * NQ],
                        y_ps[:], reps[:], MUL)

                pending = None
                for h in range(HPC):
                    inst = b * HPC + h
                    for j in range(NJ):
                        y_ps = psy.tile([P, NQ], F32, tag="y",
                                        name=f"y_{inst}_{j}")
                        s_ps = psm.tile([1, NQ], F32, tag="s",
                                        name=f"s_{inst}_{j}")
                        sacc = saccp.tile([P, NQ], MMDT, tag="sa",
                                          name=f"sa_{inst}_{j}")
                        nkt = 4 * (j + 1)
                        for kt in range(nkt):
                            # diagonal key tiles: columns < qlo are fully
                            # masked; skip them in every op (causal
                            # narrowing saves ~15% of exp/adds/matmul)
                            ktl = kt - (nkt - 4)
                            qlo = max(ktl, 0) * P
                            sc = pss.tile([P, NQ], F32, tag="sc",
                                          name=f"sc_{inst}_{j}_{kt}")
                            nc.tensor.matmul(
                                sc[:, qlo:],
                                qkT[:, (2 + h) * T + kt * P:
                                    (2 + h) * T + (kt + 1) * P],
                                qkT[:, h * T + j * NQ + qlo:
                                    h * T + (j + 1) * NQ],
                                start=True, stop=True)
                            at = attnp.tile([P, NQ], MMDT, tag="at",
                                            name=f"at_{inst}_{j}_{kt}")
                            nc.scalar.activation(at[:, qlo:], sc[:, qlo:],
                                                 EXP, scale=SCALE)
                            if ktl >= 0:
                                # strict-triangle mask on the one partially
                                # valid 128-col subtile
                                nc.vector.tensor_tensor(
                                    at[:, qlo:qlo + P], at[:, qlo:qlo + P],
                                    msk_sb[:], MUL)
                            # denominator partials on DVE (fp16 lanes, each
                            # accumulates <=16 exp values -> no overflow)
                            if kt == 0:
                                nc.vector.tensor_copy(sacc[:], at[:])
                            else:
                                nc.vector.tensor_tensor(sacc[:, qlo:],
                                                        sacc[:, qlo:],
                                                        at[:, qlo:], ADD)
                            nc.tensor.matmul(
                                y_ps[:, qlo:],
                                v_sb[:, kt * HPC * D + h * D:
                                     kt * HPC * D + (h + 1) * D],
                                at[:, qlo:], start=(kt == 0),
                                stop=(kt == nkt - 1), skip_group_check=True)
                            if kt == 1 and pending is not None:
                                pending = (*pending[:3],
                                           finalize_a(*pending))
                            if kt == 3 and pending is not None:
                                finalize_b(*pending)
                                pending = None
                        # single partition-sum matmul per query chunk
                        nc.tensor.matmul(s_ps[:], ones_c[:], sacc[:],
                                         start=True, stop=True)
                        pending = (inst, j, y_ps, s_ps)
                if pending is not None:
                    finalize_b(*pending[:3], finalize_a(*pending))
                    pending = None

        wstack.close()

        # ---- Phase C: partial output projection ----
        with ExitStack() as cctx:
            cp = cctx.enter_context
            outrow = cp(tc.tile_pool(name="orow", bufs=3))
            pso = cp(tc.tile_pool(name="pso", bufs=8, space=PSUM))
            for b in range(B):
                for m in range(MT):
                    orow = outrow.tile([P, C], MMDT, tag="orow")
                    for oc in range(4):
                        op = pso.tile([P, 512], F32, tag="op")
                        for h in range(HPC):
                            nc.tensor.matmul(
                                op[:],
                                yt_all[:, (b * HPC + h) * T + m * P:
                                       (b * HPC + h) * T + (m + 1) * P],
                                wp_sb[:, h * C + oc * 512: h * C + (oc + 1) * 512],
                                start=(h == 0), stop=(h == HPC - 1))
                        if oc % 2 == 0:
                            nc.scalar.copy(orow[:, oc * 512:(oc + 1) * 512], op[:])
                        else:
                            nc.vector.tensor_copy(
                                orow[:, oc * 512:(oc + 1) * 512], op[:])
                    nc.sync.dma_start(
                        out=out_d[(b * MT + m) * P:(b * MT + m + 1) * P, :],
                        in_=orow[:])

    nc.compile()
    return nc


def _perm(rows):
    return np.concatenate([rows[0::2], rows[1::2]], axis=0)


def _host_inputs(x, mask, freqs_cos, freqs_sin, w_attn, w_proj):
    f32 = np.float32
    f16 = np.float16
    x = np.asarray(x, f32)
    mask = np.asarray(mask)
    fc = np.asarray(freqs_cos, f32)
    fs = np.asarray(freqs_sin, f32)
    w_attn = np.asarray(w_attn, f32)
    w_proj = np.asarray(w_proj, f32)

    # x in DMA-issue order: per partition, contiguous [b][group][half][k8][tok]
    Xv = x.reshape(B, T, KTC, P).transpose(3, 0, 2, 1)  # [p, b, kt, t]
    chunks = []
    for b in range(B):
        m0 = 0
        for g in GROUPS:
            blk = Xv[:, b, :, m0 * P:(m0 + g) * P]       # [p, 16, g*128]
            chunks.append(blk.reshape(P, -1))
            m0 += g
    xt_host = np.ascontiguousarray(np.concatenate(chunks, axis=1)).astype(f16)

    def rows_arrange(a):  # [BT, RD] -> [P, (BT//P)*RD]
        return np.ascontiguousarray(
            a.reshape(BT // P, P, RD).transpose(1, 0, 2).reshape(P, -1))

    cosw = rows_arrange(np.concatenate([fc] * B, axis=0))
    sinw = rows_arrange(np.concatenate([fs] * B, axis=0))

    # one [k, q] triangle (attend iff k <= q) covers every diagonal subtile
    maskd = np.ascontiguousarray(np.triu(np.ones((P, P), dtype=f16)))

    wq, wk, wv = w_attn[0:C], w_attn[C:2 * C], w_attn[2 * C:3 * C]
    in_maps = []
    for c in range(NCORES):
        h0, h1 = HPC * c, HPC * c + 1
        Wc = np.concatenate([
            _perm(wq[h0 * D:(h0 + 1) * D]), _perm(wq[h1 * D:(h1 + 1) * D]),
            _perm(wk[h0 * D:(h0 + 1) * D]), _perm(wk[h1 * D:(h1 + 1) * D]),
            wv[h0 * D:(h0 + 1) * D], wv[h1 * D:(h1 + 1) * D]], axis=0)
        wqkv_c = np.ascontiguousarray(
            Wc.T.reshape(KTC, P, FPC).transpose(1, 0, 2).reshape(P, KTC * FPC)
        ).astype(f16)
        wp_c = w_proj[:, c * HPC * D:(c + 1) * HPC * D].T  # [256, C]
        wp_c = np.ascontiguousarray(
            wp_c.reshape(HPC, P, C).transpose(1, 0, 2).reshape(P, HPC * C)
        ).astype(f16)
        in_maps.append({
            "xt": xt_host, "wqkv": wqkv_c, "cosw": cosw, "sinw": sinw,
            "maskd": maskd, "wproj": wp_c,
        })
    return in_maps


def kernel(x, mask, freqs_cos, freqs_sin, w_attn, w_proj):
    global _PROGRAM
    _ensure_concourse()
    from concourse.bass_utils import run_bass_kernel_spmd

    if _PROGRAM is None:
        _PROGRAM = _build_program()
    nc = _PROGRAM

    in_maps = _host_inputs(x, mask, freqs_cos, freqs_sin, w_attn, w_proj)
    res = run_bass_kernel_spmd(nc, in_maps, list(range(NCORES)))
    out = res.results[0]["outp"].astype(np.float32)
    for i in range(1, NCORES):
        out = out + res.results[i]["outp"].astype(np.float32)
    return np.ascontiguousarray(out.reshape(B, T, C))
